# revision 19
# baseline (speedup 1.0000x reference)
"""Multi-head attention with fraction-based RoPE ("stoich RoPE") on 8
Trainium2 NeuronCores.

Sharding (v4): each core owns one (batch, head-half) pair -- B=4 batches
x 2 head-halves = 8 shards.  A core projects Q/K/V for its 8 heads over
all 2048 tokens (no redundant work), runs attention for its 4 head
pairs, and computes a PARTIAL output projection (its 512 attn dims of
the 1024-dim contraction).  The host sums the two partials per batch;
the output bias is fed only to the even cores so it is added once.

Device program per core (SPMD):
  - x^T resident in SBUF (8 per-f-chunk tiles, loaded once up front).
  - per head pair (4): projections pumped into the previous pair's
    attention; RoPE via two DVE multiplies with the 32-block partition
    swap done by the DMA engines; V transposed on the PE in groups of 4
    chunks per PSUM->SBUF copy.
  - attention: the two heads' score matmuls are issued adjacently
    (contract=64 -> concurrent PE row-tiles 0/64); one [128,1024] exp
    per key chunk on ACT; PV accumulates both heads in one PSUM tile
    with a ones column for the denominators.
  - softmax normalization: bit-trick + one-Newton reciprocal (-1/den)
    on the DVE, broadcast by a K=1 f32r matmul, multiply-as-eviction;
    the sign is repaid by (bias - pout) in the output projection.
  - output projection pumped into the last pair's attention.
"""

import contextlib
import ctypes
import sys
import types

import numpy as np
import ml_dtypes

import concourse.bass as bass
import concourse.mybir as mybir
import concourse.tile as tile
from concourse.masks import make_identity
from concourse.vector_clock import ScopedClock

# ---------------- problem constants (hardcoded per contract) ----------------
B, T, D = 4, 2048, 1024
H, HD = 16, 64  # heads, head dim
HALF = HD // 2
N_CORES = 8
P = 128
NQ = 512  # moving-dim tile for matmuls
NFD = D // P  # 8 contraction chunks for the projections
NPC = 4  # head pairs per core (8 heads)
DC = D // 2  # per-core projection output width
NCH = T // P  # 16 key chunks
NQB = T // NQ  # 4 query blocks
SCALE = 1.0 / np.sqrt(HD)  # folded into exp()
ROPE_SCALE = 1000.0
ROPE_BASE = 10000.0

F32 = mybir.dt.float32
DT_MM = mybir.dt.bfloat16  # dtype of matmul operands (bfloat16 | float32)

_SO_PATH = "/opt/axon/libaxon_pjrt.so"


# ---------------- axon/NTFF environment shims ----------------
def _ntff_profile_hook():
    try:
        lib = ctypes.CDLL(_SO_PATH)
    except OSError:
        return None
    if not hasattr(lib, "axon_start_nrt_profile"):
        return None
    lib.axon_start_nrt_profile.argtypes = [
        ctypes.POINTER(ctypes.c_int64),
        ctypes.c_size_t,
    ]
    lib.axon_start_nrt_profile.restype = ctypes.c_int64
    lib.axon_stop_nrt_profile.argtypes = [ctypes.c_char_p]
    lib.axon_stop_nrt_profile.restype = ctypes.c_int64

    @contextlib.contextmanager
    def _hook(output_dir, device_ids):
        import jax

        jax.devices()
        if device_ids:
            ids = (ctypes.c_int64 * len(device_ids))(*device_ids)
            rc = lib.axon_start_nrt_profile(ids, len(device_ids))
        else:
            rc = lib.axon_start_nrt_profile(None, 0)
        if rc != 0:
            raise RuntimeError(f"axon_start_nrt_profile rc={rc}")
        try:
            yield
        finally:
            n = lib.axon_stop_nrt_profile(str(output_dir).encode())
            if n < 0:
                raise RuntimeError(f"axon_stop_nrt_profile rc={n}")

    return _hook


def install_shims():
    if "antenv.axon_hooks" not in sys.modules:
        mod = types.ModuleType("antenv.axon_hooks")
        hook = _ntff_profile_hook()
        mod.get_axon_ntff_profile_hook = lambda: hook
        mod.set_axon_ntff_profile_hook = lambda h: None
        sys.modules["antenv.axon_hooks"] = mod
    import concourse.bass_utils as bass_utils

    bass_utils.upload_artifacts = lambda tmpdir: str(tmpdir)

    import os

    if os.environ.get("BASS_LDW_OPT") == "1" and not getattr(
        bass_utils, "_ldw_opt_patched", False
    ):
        orig_run = bass_utils.run_command

        def _run_ldw(argv, **kw):
            argv = [
                "--enable-ldw-opt=true" if a == "--enable-ldw-opt=false" else a
                for a in argv
            ]
            return orig_run(argv, **kw)

        bass_utils.run_command = _run_ldw
        bass_utils._ldw_opt_patched = True


class TileContextSplitDrain(tile.TileContext):
    """This walrus build encodes at most 2 sync waits per CTRL
    instruction; Tile's kernel-tail drain wants one wait per logical
    processor.  Split the waits across single-wait NOPs instead."""

    MAX_WAITS = 1

    def _drain_and_barrier(self, tick_clock, wait_clock):
        nc = self.nc
        carrier = nc.sync.nop(nofuse=True)
        wait_clock.add_sem_waits(
            carrier.ins, ScopedClock({None: tick_clock.global_clock})
        )
        waits = list(carrier.ins.sync_info.on_wait or [])
        if len(waits) > self.MAX_WAITS:
            carrier.ins.sync_info.on_wait[:] = waits[: self.MAX_WAITS]
            for i in range(self.MAX_WAITS, len(waits), self.MAX_WAITS):
                extra = nc.sync.nop(nofuse=True)
                extra.ins.sync_info = mybir.SyncInfo(
                    on_wait=list(waits[i : i + self.MAX_WAITS]), on_update=[]
                )
        nc.sync.drain()
        nc.all_engine_barrier()
        assert self.sems is not None
        popped = nc._tile_sem_poison_stack.pop()
        assert popped is self._sem_poison
        nc.clear_and_free_semaphores(list(self.sems.allocated().values()))
        nc.all_engine_barrier()


def _split_sync_waits(nc, max_waits=1):
    """This walrus build rejects instructions carrying more than ~2 sync
    waits.  Move excess waits onto same-engine NOPs inserted just before
    the instruction (AND semantics are preserved: the engine blocks on
    each carrier in program order)."""
    for f in nc.m.functions:
        for bb in f.blocks:
            out = []
            for inst in bb.instructions:
                si = inst.sync_info
                waits = list(si.on_wait) if si and si.on_wait else []
                if len(waits) > max_waits:
                    for i in range(0, len(waits) - max_waits, max_waits):
                        nop = mybir.InstNoOp(
                            name=nc.get_next_instruction_name(), ins=[], outs=[]
                        )
                        nop.engine = inst.engine
                        nop.sync_info = mybir.SyncInfo(
                            on_wait=list(waits[i : i + max_waits]), on_update=[]
                        )
                        nc.register_instruction(nop, overwrite=True)
                        out.append(nop)
                    si.on_wait[:] = waits[len(waits) - max_waits :]
                out.append(inst)
            bb.instructions[:] = out


# ---------------- device program ----------------
def build_nc(dt_mm=DT_MM):
    nc = bass.Bass(
        "TRN2", target_bir_lowering=False, debug=False, num_devices=N_CORES
    )

    xt = nc.dram_tensor("xt", [D, T], dt_mm, kind="ExternalInput")
    wqt = nc.dram_tensor("wqt", [D, DC], dt_mm, kind="ExternalInput")
    wkt = nc.dram_tensor("wkt", [D, DC], dt_mm, kind="ExternalInput")
    wvt = nc.dram_tensor("wvt", [D, DC], dt_mm, kind="ExternalInput")
    wot = nc.dram_tensor("wot", [DC, D], dt_mm, kind="ExternalInput")
    bq = nc.dram_tensor("bq", [P, NPC], F32, kind="ExternalInput")
    bk = nc.dram_tensor("bk", [P, NPC], F32, kind="ExternalInput")
    bv = nc.dram_tensor("bv", [P, NPC], F32, kind="ExternalInput")
    bob = nc.dram_tensor("bob", [P, D], F32, kind="ExternalInput")
    csak = nc.dram_tensor("csak", [P, T], dt_mm, kind="ExternalInput")
    csbk = nc.dram_tensor("csbk", [P, T], dt_mm, kind="ExternalInput")
    out = nc.dram_tensor("out", [T, D], F32, kind="ExternalOutput")

    AF = mybir.ActivationFunctionType

    with TileContextSplitDrain(nc) as tc:
        persist_cm = tc.tile_pool(name="persist", bufs=1)
        persist = persist_cm.__enter__()

        def ptile(shape, dt, tag):
            return persist.tile(shape, dt, tag=tag, name=tag)

        with contextlib.ExitStack() as ctx:
            # ---- persistent tiles ----
            # resident x^T, one tile per contraction chunk so the first
            # projection can start as soon as chunk 0 lands
            xt_ts = [ptile([P, T], dt_mm, f"xt{f}") for f in range(NFD)]
            csak_t = ptile([P, T], dt_mm, "csak_t")
            csbk_t = ptile([P, T], dt_mm, "csbk_t")
            bq_t = ptile([P, NPC], F32, "bq_t")
            bk_t = ptile([P, NPC], F32, "bk_t")
            bv_t = ptile([P, NPC], F32, "bv_t")
            ident = ptile([P, HD], dt_mm, "ident")
            ones64_f = ptile([1, HD], F32, "ones64_f")
            ones64r = ptile([1, HD], mybir.dt.float32r, "ones64r")
            attn = [ptile([P, T], dt_mm, f"attn{pr}") for pr in range(NPC)]
            for f in range(NFD):
                nc.sync.dma_start(xt_ts[f][:], xt[f * P : (f + 1) * P, :])
            nc.sync.dma_start(csak_t[:], csak[:])
            nc.sync.dma_start(csbk_t[:], csbk[:])
            nc.sync.dma_start(bq_t[:], bq[:])
            nc.sync.dma_start(bk_t[:], bk[:])
            nc.sync.dma_start(bv_t[:], bv[:])
            make_identity(nc, ident[0:HD, :])
            make_identity(nc, ident[HD : 2 * HD, :])
            # +1: the Newton chain yields -1/den, so pb = -1/den and the
            # attn tiles carry -attn/den; the output projection's
            # (bias - pout) restores the sign
            nc.vector.memset(ones64_f[:], 1.0)
            with nc.allow_low_precision(reason="f32r ones for rec bcast"):
                nc.scalar.copy(ones64r[:], ones64_f[:])

            # ---- pools for the head-pair loop ----
            wp = ctx.enter_context(tc.tile_pool(name="wp", bufs=2))
            rawp = ctx.enter_context(tc.tile_pool(name="rawp", bufs=1))
            ropep = ctx.enter_context(tc.tile_pool(name="ropep", bufs=1))
            vtp = ctx.enter_context(tc.tile_pool(name="vtp", bufs=1))
            qkp = ctx.enter_context(tc.tile_pool(name="qkp", bufs=2))
            vnp = ctx.enter_context(tc.tile_pool(name="vnp", bufs=2))
            exp_p = ctx.enter_context(tc.tile_pool(name="exp_p", bufs=4))
            smallp = ctx.enter_context(tc.tile_pool(name="smallp", bufs=2))
            normp = ctx.enter_context(tc.tile_pool(name="normp", bufs=1))
            h1p = ctx.enter_context(tc.tile_pool(name="h1p", bufs=2))
            ps_proj = ctx.enter_context(
                tc.tile_pool(name="ps_proj", bufs=2, space="PSUM")
            )
            ps_sc = ctx.enter_context(
                tc.tile_pool(name="ps_sc", bufs=2, space="PSUM")
            )
            ps_po = ctx.enter_context(
                tc.tile_pool(name="ps_po", bufs=1, space="PSUM")
            )

            def rope(raw, raws, csa_t, csb_t, out_tile):
                # out = raw*csa + swap32(raw)*csb.  The 32-block partition
                # swap is done by the DMA engines (raws), freeing the DVE.
                m1 = ropep.tile([P, T], dt_mm, tag="m1", name="m1")
                m2 = ropep.tile([P, T], dt_mm, tag="m2", name="m2")
                nc.vector.tensor_mul(m1[:], raw[:], csa_t[:])
                nc.vector.tensor_mul(m2[:], raws[:], csb_t[:])
                nc.vector.tensor_add(out_tile[:], m1[:], m2[:])

            def dma_swap32(dst, src_t):
                # dst[32-block swapped within each 64-block] = src
                for blk in range(2):
                    b0 = blk * 64
                    nc.sync.dma_start(
                        dst[b0 : b0 + 32, :], src_t[b0 + 32 : b0 + 64, :]
                    )
                    nc.sync.dma_start(
                        dst[b0 + 32 : b0 + 64, :], src_t[b0 : b0 + 32, :]
                    )

            def stage_units(pr):
                """Emission units for pair pr's projections + RoPE + V
                transpose.  Each unit emits a small instruction group; the
                attention loop of the previous pair pumps these so the PE
                stays dense while ACT works on exp."""
                d0 = pr * P
                st = {}
                units = []

                def u_wdma():
                    st["wq"] = wp.tile([P, NFD, P], dt_mm, tag="wq", name="wq_c")
                    st["wk"] = wp.tile([P, NFD, P], dt_mm, tag="wk", name="wk_c")
                    st["wv"] = wp.tile([P, NFD, P], dt_mm, tag="wv", name="wv_c")
                    for key, w in (("wq", wqt), ("wk", wkt), ("wv", wvt)):
                        nc.sync.dma_start(
                            st[key][:],
                            w[:, d0 : d0 + P].rearrange("(f p) d -> p f d", p=P),
                        )
                    st["qraw"] = rawp.tile([P, T], dt_mm, tag="qraw", name="q_raw")
                    st["kraw"] = rawp.tile([P, T], dt_mm, tag="kraw", name="k_raw")
                    st["qraws"] = rawp.tile([P, T], dt_mm, tag="qraws", name="q_raws")
                    st["kraws"] = rawp.tile([P, T], dt_mm, tag="kraws", name="k_raws")
                    st["vt"] = vtp.tile([P, T], dt_mm, tag="vt", name="v_t")

                units.append(u_wdma)

                def u_mm(w_key, nb, f, start, stop):
                    def go():
                        if start:
                            st["ps"] = ps_proj.tile([P, NQ], F32, tag="ps", name="ps")
                        nc.tensor.matmul(
                            st["ps"][:],
                            st[w_key][:, f, :],
                            xt_ts[f][:, nb * NQ : (nb + 1) * NQ],
                            start=start,
                            stop=stop,
                        )

                    return go

                def u_evict(b_t, dst_key, dslice):
                    def go():
                        nc.vector.tensor_scalar_add(
                            st[dst_key][:, dslice],
                            st["ps"][:],
                            b_t[:, pr : pr + 1],
                        )

                    return go

                def u_swap(rkey, skey, nb):
                    def go():
                        sl = slice(nb * NQ, (nb + 1) * NQ)
                        dma_swap32(st[skey][:, sl], st[rkey][:, sl])

                    return go

                for nb in range(T // NQ):
                    for w_key, b_t, dst_key in (
                        ("wk", bk_t, "kraw"),
                        ("wv", bv_t, "vt"),
                        ("wq", bq_t, "qraw"),
                    ):
                        for f in range(NFD):
                            units.append(
                                u_mm(w_key, nb, f, f == 0, f == NFD - 1)
                            )
                        units.append(
                            u_evict(b_t, dst_key, slice(nb * NQ, (nb + 1) * NQ))
                        )
                        if dst_key != "vt":
                            units.append(
                                u_swap(dst_key, dst_key + "s", nb)
                            )

                def u_rope():
                    st["qt"] = qkp.tile([P, T], dt_mm, tag="qt", name="qt")
                    rope(st["qraw"], st["qraws"], csak_t, csbk_t, st["qt"])

                def u_rope2():
                    st["kt"] = qkp.tile([P, T], dt_mm, tag="kt", name="kt")
                    rope(st["kraw"], st["kraws"], csak_t, csbk_t, st["kt"])

                units.append(u_rope)
                units.append(u_rope2)

                def u_vn_alloc(hh):
                    def go():
                        vn_h = vnp.tile(
                            [P, NCH, HD + 1], dt_mm, tag=f"vn{hh}", name="vn_h"
                        )
                        # col 64 = ones (softmax denominator)
                        nc.vector.memset(vn_h[:, :, HD : HD + 1], 1.0)
                        st[f"vn{hh}"] = vn_h

                    return go

                def u_vtr4(hh, g):
                    # transpose 4 key chunks of head hh, one PSUM->SBUF copy
                    def go():
                        tp = ps_proj.tile([P, 4, HD], dt_mm, tag="ps", name="tp")
                        h0 = hh * HD
                        for k in range(4):
                            ch = g * 4 + k
                            nc.tensor.transpose(
                                tp[:, k, :],
                                st["vt"][h0 : h0 + HD, ch * P : (ch + 1) * P],
                                ident[h0 : h0 + HD, :],
                            )
                        nc.vector.tensor_copy(
                            st[f"vn{hh}"][:, g * 4 : g * 4 + 4, :HD], tp[:]
                        )

                    return go

                for hh in range(2):
                    units.append(u_vn_alloc(hh))
                    for g in range(NCH // 4):
                        units.append(u_vtr4(hh, g))
                return st, units

            def pump(units, n):
                for _ in range(n):
                    if units:
                        units.pop(0)()

            def attention(pr, st, next_units, pump_rate, carry_norm,
                          late_units=None):
                """Attention for pair pr using st['qt'/'kt'/'vn*'].  The two
                heads' score matmuls are issued adjacently (concurrent PE
                row-tiles).  po packs both heads' PV accumulation per
                query-block; normalization: bit-trick + one-Newton
                reciprocal (DVE), K=1 f32r broadcast matmul, multiply-as-
                eviction.  Returns the deferred normalize closure for the
                last query block (run by the next pair's attention)."""
                attn_h1 = h1p.tile([HD, T], dt_mm, tag="h1", name="attn_h1")

                def make_norm(po, qb):
                    qs = slice(qb * NQ, (qb + 1) * NQ)
                    den = po[HD : HD + 1, :, :]
                    sd = normp.tile([1, 2, NQ], mybir.dt.int32, tag="sd", name="sd")
                    nc.vector.tensor_scalar(
                        sd[:],
                        den.bitcast(mybir.dt.int32),
                        -1,
                        0x7EF311C3,
                        mybir.AluOpType.mult,
                        mybir.AluOpType.add,
                    )
                    y0 = sd[:].bitcast(F32)
                    t0 = normp.tile([1, 2, NQ], F32, tag="t0", name="t0")
                    nc.vector.tensor_mul(t0[:], den, y0)
                    rec = normp.tile(
                        [1, 2, NQ], mybir.dt.float32r, tag="rec", name="rec"
                    )
                    with nc.allow_low_precision(reason="f32r rec for bcast mm"):
                        nc.vector.scalar_tensor_tensor(
                            rec[:], t0[:], -2.0, y0,
                            mybir.AluOpType.add, mybir.AluOpType.mult,
                        )

                    def bcast_mul():
                        for hh in range(2):
                            pb = ps_sc.tile([HD, NQ], F32, tag="sc", name="pb")
                            nc.tensor.matmul(
                                pb[:],
                                ones64r[:],
                                rec[0:1, hh, :],
                                start=True,
                                stop=True,
                            )
                            recb = smallp.tile(
                                [HD, NQ], F32, tag=f"recb{hh}", name="recb"
                            )
                            nc.vector.tensor_copy(recb[:], pb[:])
                            dst = attn[pr][0:HD, qs] if hh == 0 else attn_h1[:, qs]
                            nc.vector.tensor_mul(
                                dst, po[0:HD, hh, :], recb[:]
                            )
                        # odd head into the pair region per query block so
                        # attn[pr] completes incrementally (cross-quadrant
                        # 32-partition copies)
                        nc.vector.tensor_copy(
                            attn[pr][64:96, qs], attn_h1[0:32, qs]
                        )
                        nc.vector.tensor_copy(
                            attn[pr][96:128, qs], attn_h1[32:64, qs]
                        )

                    return bcast_mul

                for qb in range(NQB):
                    qs = slice(qb * NQ, (qb + 1) * NQ)
                    po = ps_po.tile([P, 2, NQ], F32, tag="po", name="po")
                    pending_pv = None
                    for ci in range(NCH):
                        ps2 = ps_sc.tile([P, 2 * NQ], F32, tag="sc", name="ps2")
                        for hh in range(2):
                            h0 = hh * HD
                            nc.tensor.matmul(
                                ps2[:, hh * NQ : (hh + 1) * NQ],
                                st["kt"][h0 : h0 + HD, ci * P : (ci + 1) * P],
                                st["qt"][h0 : h0 + HD, qs],
                                start=True,
                                stop=True,
                            )
                        pexp = exp_p.tile([P, 2 * NQ], dt_mm, tag="ex", name="pexp")
                        nc.scalar.activation(
                            pexp[:], ps2[:], AF.Exp, scale=float(SCALE)
                        )
                        pump(next_units, pump_rate)
                        if ci == 1 and carry_norm is not None:
                            carry_norm()
                            carry_norm = None
                        if late_units and ci >= 2 and late_units[0][0] < qb:
                            late_units.pop(0)[1]()
                        # PV runs one iteration behind so exp latency is hidden
                        if pending_pv is not None:
                            pending_pv()

                        def make_pv(pexp=pexp, ci=ci, po=po):
                            def go():
                                for hh in range(2):
                                    nc.tensor.matmul(
                                        po[0 : HD + 1, hh, :],
                                        st[f"vn{hh}"][:, ci, :],
                                        pexp[:, hh * NQ : (hh + 1) * NQ],
                                        start=(ci == 0),
                                        stop=(ci == NCH - 1),
                                    )

                            return go

                        pending_pv = make_pv()
                    pending_pv()
                    norm = make_norm(po, qb)
                    if qb < NQB - 1:
                        carry_norm = norm

                return norm

            # ---- output projection resources (prefetched up front; the
            # matmuls are pumped into the last pair's attention) ----
            wop = ctx.enter_context(tc.tile_pool(name="wop", bufs=1))
            outp = ctx.enter_context(tc.tile_pool(name="outp", bufs=2))
            bob_t = persist.tile([P, D], F32, tag="bob_t", name="bob_t")
            nc.sync.dma_start(bob_t[:], bob[:])
            wo_c = []
            for ch in range(NPC):
                wo_ch = wop.tile([P, D], dt_mm, tag=f"wo{ch}", name="wo_ch")
                nc.sync.dma_start(wo_ch[:], wot[ch * P : (ch + 1) * P, :])
                wo_c.append(wo_ch)

            osb_of = {}

            def u_outproj(tb, nh):
                # half an output row-block: 4 accumulating matmuls + bias-sub
                def go():
                    ts = slice(tb * P, (tb + 1) * P)
                    pout = ps_proj.tile([P, NQ], F32, tag="ps", name="pout")
                    for ch in range(NPC):
                        nc.tensor.matmul(
                            pout[:],
                            attn[ch][:, ts],
                            wo_c[ch][:, nh * NQ : (nh + 1) * NQ],
                            start=(ch == 0),
                            stop=(ch == NPC - 1),
                        )
                    if tb not in osb_of:
                        osb_of[tb] = outp.tile([P, D], F32, tag="osb", name="osb")
                    osb = osb_of[tb]
                    # attn tiles carry -attn/den (sign from the Newton
                    # chain); bias - pout restores the sign for free
                    nc.vector.tensor_sub(
                        osb[:, nh * NQ : (nh + 1) * NQ],
                        bob_t[:, nh * NQ : (nh + 1) * NQ],
                        pout[:],
                    )
                    if nh == 1:
                        nc.sync.dma_start(out[ts, :], osb[:])

                return go

            st, units = stage_units(0)
            pump(units, len(units))
            carry_tail = None
            # out-proj row-blocks gated on the attn[NPC-1] query block they
            # read (tb//4); pumped into the last pair's later query blocks
            late_units = [
                (tb // (NQ // P), u_outproj(tb, nh))
                for tb in range(3 * T // P // 4)
                for nh in range(2)
            ]
            for pr in range(NPC):
                if pr + 1 < NPC:
                    nxt_st, nxt_units = stage_units(pr + 1)
                else:
                    nxt_st, nxt_units = None, []
                pump_rate = (len(nxt_units) + 63) // 64 + 1 if nxt_units else 0
                carry_tail = attention(
                    pr, st, nxt_units, pump_rate, carry_tail,
                    late_units=late_units if pr == NPC - 1 else None,
                )
                pump(nxt_units, len(nxt_units))
                st = nxt_st
            carry_tail()
            for _, u in late_units:
                u()
            for tb in range(3 * T // P // 4, T // P):
                for nh in range(2):
                    u_outproj(tb, nh)()

        persist_cm.__exit__(None, None, None)

    _split_sync_waits(nc)
    return nc


# ---------------- host-side input prep ----------------
def _np_dt(dt_mm):
    return ml_dtypes.bfloat16 if dt_mm == mybir.dt.bfloat16 else np.float32


def _cs_tiles(frac_b):
    """csa/csb [128, T] f32 RoPE tiles for one batch (frac_b: [T] f32)."""
    i = np.arange(HALF, dtype=np.float64)
    freq = (ROPE_BASE ** (2.0 * i / HD)).astype(np.float32)  # [32]
    pos = frac_b.astype(np.float32) * np.float32(ROPE_SCALE)
    ang = pos[None, :] / freq[:, None]  # [32, T] f32
    a64 = ang.astype(np.float64)
    cos = np.cos(a64).astype(np.float32)
    sin = np.sin(a64).astype(np.float32)
    csa = np.tile(cos, (4, 1))  # [128, T]
    # csb multiplies the pre-swapped raw (raws[p] = raw[swap32(p)]), so the
    # sign lives at the OUTPUT row: rows 0-31 pair with xr and need -sin,
    # rows 32-63 pair with xl and need +sin
    csb = np.tile(np.concatenate([-sin, sin], axis=0), (2, 1))  # [128, T]
    return np.ascontiguousarray(csa), np.ascontiguousarray(csb)


def make_in_maps(x, frac, Wq, bq, Wk, bk, Wv, bv, Wo, bo, dt_mm=DT_MM):
    npdt = _np_dt(dt_mm)
    wqt = np.ascontiguousarray(Wq.T).astype(npdt)  # [D_in, D_out]
    wkt = np.ascontiguousarray(Wk.T).astype(npdt)
    wvt = np.ascontiguousarray(Wv.T).astype(npdt)
    wot = np.ascontiguousarray(Wo.T).astype(npdt)  # [attn_dim, D_out]
    bq_p = bq.reshape(NFD, P).T.astype(np.float32)  # [128, 8 pairs]
    bk_p = bk.reshape(NFD, P).T.astype(np.float32)
    bv_p = bv.reshape(NFD, P).T.astype(np.float32)
    bob = np.ascontiguousarray(np.tile(bo[None, :], (P, 1))).astype(np.float32)
    zeros_bob = np.zeros_like(bob)
    in_maps = []
    for c in range(N_CORES):
        b, hh2 = c // 2, c % 2
        cols = slice(hh2 * DC, (hh2 + 1) * DC)
        prs = slice(hh2 * NPC, (hh2 + 1) * NPC)
        xt_c = np.ascontiguousarray(x[b].T).astype(npdt)  # [D, T]
        csa, csb = _cs_tiles(frac[b])
        in_maps.append(
            {
                "xt": xt_c,
                "wqt": np.ascontiguousarray(wqt[:, cols]),
                "wkt": np.ascontiguousarray(wkt[:, cols]),
                "wvt": np.ascontiguousarray(wvt[:, cols]),
                "wot": np.ascontiguousarray(wot[cols, :]),
                "bq": np.ascontiguousarray(bq_p[:, prs]),
                "bk": np.ascontiguousarray(bk_p[:, prs]),
                "bv": np.ascontiguousarray(bv_p[:, prs]),
                # bias fed once per batch (even core); odd core gets zeros
                "bob": bob if hh2 == 0 else zeros_bob,
                "csak": csa.astype(npdt),
                "csbk": csb.astype(npdt),
            }
        )
    return in_maps


_NC_CACHE = {}


def _get_nc(dt_mm=DT_MM):
    key = str(dt_mm)
    if key not in _NC_CACHE:
        _NC_CACHE[key] = build_nc(dt_mm)
    return _NC_CACHE[key]


def kernel(x, frac, Wq, bq, Wk, bk, Wv, bv, Wo, bo):
    install_shims()
    from concourse.bass_utils import run_bass_kernel_spmd

    x = np.asarray(x, dtype=np.float32)
    frac = np.asarray(frac, dtype=np.float32)
    args = [np.asarray(a, dtype=np.float32) for a in (Wq, bq, Wk, bk, Wv, bv, Wo, bo)]
    in_maps = make_in_maps(x, frac, *args, dt_mm=DT_MM)
    nc = _get_nc(DT_MM)
    res = run_bass_kernel_spmd(nc, in_maps, list(range(N_CORES)))
    out = np.empty((B, T, D), dtype=np.float32)
    for b in range(B):
        out[b] = res.results[2 * b]["out"] + res.results[2 * b + 1]["out"]
    return out


# revision 20
# speedup vs baseline: 1.0672x; 1.0672x over previous
"""Multi-head attention with fraction-based RoPE ("stoich RoPE") on 8
Trainium2 NeuronCores.

Sharding (v4): each core owns one (batch, head-half) pair -- B=4 batches
x 2 head-halves = 8 shards.  A core projects Q/K/V for its 8 heads over
all 2048 tokens (no redundant work), runs attention for its 4 head
pairs, and computes a PARTIAL output projection (its 512 attn dims of
the 1024-dim contraction).  The host sums the two partials per batch;
the output bias is fed only to the even cores so it is added once.

Device program per core (SPMD):
  - x^T resident in SBUF (8 per-f-chunk tiles, loaded once up front).
  - per head pair (4): projections pumped into the previous pair's
    attention; RoPE via two DVE multiplies with the 32-block partition
    swap done by the DMA engines; V transposed on the PE in groups of 4
    chunks per PSUM->SBUF copy.
  - attention: the two heads' score matmuls are issued adjacently
    (contract=64 -> concurrent PE row-tiles 0/64); one [128,1024] exp
    per key chunk on ACT; PV accumulates both heads in one PSUM tile
    with a ones column for the denominators.
  - softmax normalization: bit-trick + one-Newton reciprocal (-1/den)
    on the DVE, broadcast by a K=1 f32r matmul, multiply-as-eviction;
    the sign is repaid by (bias - pout) in the output projection.
  - output projection pumped into the last pair's attention.
"""

import contextlib
import ctypes
import sys
import types

import numpy as np
import ml_dtypes

import concourse.bass as bass
import concourse.mybir as mybir
import concourse.tile as tile
from concourse.masks import make_identity
from concourse.vector_clock import ScopedClock

# ---------------- problem constants (hardcoded per contract) ----------------
B, T, D = 4, 2048, 1024
H, HD = 16, 64  # heads, head dim
HALF = HD // 2
N_CORES = 8
P = 128
NQ = 512  # moving-dim tile for matmuls
NFD = D // P  # 8 contraction chunks for the projections
NPC = 4  # head pairs per core (8 heads)
DC = D // 2  # per-core projection output width
NCH = T // P  # 16 key chunks
NQB = T // NQ  # 4 query blocks
SCALE = 1.0 / np.sqrt(HD)  # folded into exp()
ROPE_SCALE = 1000.0
ROPE_BASE = 10000.0

F32 = mybir.dt.float32
DT_MM = mybir.dt.bfloat16  # dtype of matmul operands (bfloat16 | float32)

_SO_PATH = "/opt/axon/libaxon_pjrt.so"


# ---------------- axon/NTFF environment shims ----------------
def _ntff_profile_hook():
    try:
        lib = ctypes.CDLL(_SO_PATH)
    except OSError:
        return None
    if not hasattr(lib, "axon_start_nrt_profile"):
        return None
    lib.axon_start_nrt_profile.argtypes = [
        ctypes.POINTER(ctypes.c_int64),
        ctypes.c_size_t,
    ]
    lib.axon_start_nrt_profile.restype = ctypes.c_int64
    lib.axon_stop_nrt_profile.argtypes = [ctypes.c_char_p]
    lib.axon_stop_nrt_profile.restype = ctypes.c_int64

    @contextlib.contextmanager
    def _hook(output_dir, device_ids):
        import jax

        jax.devices()
        if device_ids:
            ids = (ctypes.c_int64 * len(device_ids))(*device_ids)
            rc = lib.axon_start_nrt_profile(ids, len(device_ids))
        else:
            rc = lib.axon_start_nrt_profile(None, 0)
        if rc != 0:
            raise RuntimeError(f"axon_start_nrt_profile rc={rc}")
        try:
            yield
        finally:
            n = lib.axon_stop_nrt_profile(str(output_dir).encode())
            if n < 0:
                raise RuntimeError(f"axon_stop_nrt_profile rc={n}")

    return _hook


def install_shims():
    if "antenv.axon_hooks" not in sys.modules:
        mod = types.ModuleType("antenv.axon_hooks")
        hook = _ntff_profile_hook()
        mod.get_axon_ntff_profile_hook = lambda: hook
        mod.set_axon_ntff_profile_hook = lambda h: None
        sys.modules["antenv.axon_hooks"] = mod
    import concourse.bass_utils as bass_utils

    bass_utils.upload_artifacts = lambda tmpdir: str(tmpdir)

    import os

    if os.environ.get("BASS_LDW_OPT") == "1" and not getattr(
        bass_utils, "_ldw_opt_patched", False
    ):
        orig_run = bass_utils.run_command

        def _run_ldw(argv, **kw):
            argv = [
                "--enable-ldw-opt=true" if a == "--enable-ldw-opt=false" else a
                for a in argv
            ]
            return orig_run(argv, **kw)

        bass_utils.run_command = _run_ldw
        bass_utils._ldw_opt_patched = True


class TileContextSplitDrain(tile.TileContext):
    """This walrus build encodes at most 2 sync waits per CTRL
    instruction; Tile's kernel-tail drain wants one wait per logical
    processor.  Split the waits across single-wait NOPs instead."""

    MAX_WAITS = 1

    def _drain_and_barrier(self, tick_clock, wait_clock):
        nc = self.nc
        carrier = nc.sync.nop(nofuse=True)
        wait_clock.add_sem_waits(
            carrier.ins, ScopedClock({None: tick_clock.global_clock})
        )
        waits = list(carrier.ins.sync_info.on_wait or [])
        if len(waits) > self.MAX_WAITS:
            carrier.ins.sync_info.on_wait[:] = waits[: self.MAX_WAITS]
            for i in range(self.MAX_WAITS, len(waits), self.MAX_WAITS):
                extra = nc.sync.nop(nofuse=True)
                extra.ins.sync_info = mybir.SyncInfo(
                    on_wait=list(waits[i : i + self.MAX_WAITS]), on_update=[]
                )
        nc.sync.drain()
        nc.all_engine_barrier()
        assert self.sems is not None
        popped = nc._tile_sem_poison_stack.pop()
        assert popped is self._sem_poison
        nc.clear_and_free_semaphores(list(self.sems.allocated().values()))
        nc.all_engine_barrier()


def _split_sync_waits(nc, max_waits=1):
    """This walrus build rejects instructions carrying more than ~2 sync
    waits.  Move excess waits onto same-engine NOPs inserted just before
    the instruction (AND semantics are preserved: the engine blocks on
    each carrier in program order)."""
    for f in nc.m.functions:
        for bb in f.blocks:
            out = []
            for inst in bb.instructions:
                si = inst.sync_info
                waits = list(si.on_wait) if si and si.on_wait else []
                if len(waits) > max_waits:
                    for i in range(0, len(waits) - max_waits, max_waits):
                        nop = mybir.InstNoOp(
                            name=nc.get_next_instruction_name(), ins=[], outs=[]
                        )
                        nop.engine = inst.engine
                        nop.sync_info = mybir.SyncInfo(
                            on_wait=list(waits[i : i + max_waits]), on_update=[]
                        )
                        nc.register_instruction(nop, overwrite=True)
                        out.append(nop)
                    si.on_wait[:] = waits[len(waits) - max_waits :]
                out.append(inst)
            bb.instructions[:] = out


# ---------------- device program ----------------
def build_nc(dt_mm=DT_MM):
    nc = bass.Bass(
        "TRN2", target_bir_lowering=False, debug=False, num_devices=N_CORES
    )

    xt = nc.dram_tensor("xt", [D, T], dt_mm, kind="ExternalInput")
    wqt = nc.dram_tensor("wqt", [D, DC], dt_mm, kind="ExternalInput")
    wkt = nc.dram_tensor("wkt", [D, DC], dt_mm, kind="ExternalInput")
    wvt = nc.dram_tensor("wvt", [D, DC], dt_mm, kind="ExternalInput")
    wot = nc.dram_tensor("wot", [DC, D], dt_mm, kind="ExternalInput")
    bq = nc.dram_tensor("bq", [P, NPC], F32, kind="ExternalInput")
    bk = nc.dram_tensor("bk", [P, NPC], F32, kind="ExternalInput")
    bv = nc.dram_tensor("bv", [P, NPC], F32, kind="ExternalInput")
    bob = nc.dram_tensor("bob", [P, D], F32, kind="ExternalInput")
    csak = nc.dram_tensor("csak", [P, T], dt_mm, kind="ExternalInput")
    csbk = nc.dram_tensor("csbk", [P, T], dt_mm, kind="ExternalInput")
    out = nc.dram_tensor("out", [T, D], F32, kind="ExternalOutput")

    AF = mybir.ActivationFunctionType

    with TileContextSplitDrain(nc) as tc:
        persist_cm = tc.tile_pool(name="persist", bufs=1)
        persist = persist_cm.__enter__()

        def ptile(shape, dt, tag):
            return persist.tile(shape, dt, tag=tag, name=tag)

        with contextlib.ExitStack() as ctx:
            # ---- persistent tiles ----
            # resident x^T, one tile per contraction chunk so the first
            # projection can start as soon as chunk 0 lands
            xt_ts = [ptile([P, T], dt_mm, f"xt{f}") for f in range(NFD)]
            csak_t = ptile([P, T], dt_mm, "csak_t")
            csbk_t = ptile([P, T], dt_mm, "csbk_t")
            bq_t = ptile([P, NPC], F32, "bq_t")
            bk_t = ptile([P, NPC], F32, "bk_t")
            bv_t = ptile([P, NPC], F32, "bv_t")
            ident = ptile([P, HD], dt_mm, "ident")
            ones64_f = ptile([1, HD], F32, "ones64_f")
            ones64r = ptile([1, HD], mybir.dt.float32r, "ones64r")
            attn = [ptile([P, T], dt_mm, f"attn{pr}") for pr in range(NPC)]
            for f in range(NFD):
                nc.sync.dma_start(xt_ts[f][:], xt[f * P : (f + 1) * P, :])
            nc.sync.dma_start(csak_t[:], csak[:])
            nc.sync.dma_start(csbk_t[:], csbk[:])
            nc.sync.dma_start(bq_t[:], bq[:])
            nc.sync.dma_start(bk_t[:], bk[:])
            nc.sync.dma_start(bv_t[:], bv[:])
            make_identity(nc, ident[0:HD, :])
            make_identity(nc, ident[HD : 2 * HD, :])
            # +1: the Newton chain yields -1/den, so pb = -1/den and the
            # attn tiles carry -attn/den; the output projection's
            # (bias - pout) restores the sign
            nc.vector.memset(ones64_f[:], 1.0)
            with nc.allow_low_precision(reason="f32r ones for rec bcast"):
                nc.scalar.copy(ones64r[:], ones64_f[:])

            # ---- pools for the head-pair loop ----
            wp = ctx.enter_context(tc.tile_pool(name="wp", bufs=2))
            rawp = ctx.enter_context(tc.tile_pool(name="rawp", bufs=1))
            ropep = ctx.enter_context(tc.tile_pool(name="ropep", bufs=1))
            vtp = ctx.enter_context(tc.tile_pool(name="vtp", bufs=1))
            qkp = ctx.enter_context(tc.tile_pool(name="qkp", bufs=2))
            vnp = ctx.enter_context(tc.tile_pool(name="vnp", bufs=2))
            exp_p = ctx.enter_context(tc.tile_pool(name="exp_p", bufs=4))
            smallp = ctx.enter_context(tc.tile_pool(name="smallp", bufs=2))
            normp = ctx.enter_context(tc.tile_pool(name="normp", bufs=1))
            h1p = ctx.enter_context(tc.tile_pool(name="h1p", bufs=2))
            ps_proj = ctx.enter_context(
                tc.tile_pool(name="ps_proj", bufs=2, space="PSUM")
            )
            ps_sc = ctx.enter_context(
                tc.tile_pool(name="ps_sc", bufs=2, space="PSUM")
            )
            ps_po = ctx.enter_context(
                tc.tile_pool(name="ps_po", bufs=1, space="PSUM")
            )

            def rope(raw, raws, csa_t, csb_t, out_tile):
                # out = raw*csa + swap32(raw)*csb.  The 32-block partition
                # swap is done by the DMA engines (raws), freeing the DVE.
                m1 = ropep.tile([P, T], dt_mm, tag="m1", name="m1")
                m2 = ropep.tile([P, T], dt_mm, tag="m2", name="m2")
                nc.vector.tensor_mul(m1[:], raw[:], csa_t[:])
                nc.vector.tensor_mul(m2[:], raws[:], csb_t[:])
                nc.vector.tensor_add(out_tile[:], m1[:], m2[:])

            def dma_swap32(dst, src_t):
                # dst[32-block swapped within each 64-block] = src
                for blk in range(2):
                    b0 = blk * 64
                    nc.sync.dma_start(
                        dst[b0 : b0 + 32, :], src_t[b0 + 32 : b0 + 64, :]
                    )
                    nc.sync.dma_start(
                        dst[b0 + 32 : b0 + 64, :], src_t[b0 : b0 + 32, :]
                    )

            def stage_units(pr):
                """Emission units for pair pr's projections + RoPE + V
                transpose.  Each unit emits a small instruction group; the
                attention loop of the previous pair pumps these so the PE
                stays dense while ACT works on exp."""
                d0 = pr * P
                st = {}
                units = []

                def u_wdma():
                    st["wq"] = wp.tile([P, NFD, P], dt_mm, tag="wq", name="wq_c")
                    st["wk"] = wp.tile([P, NFD, P], dt_mm, tag="wk", name="wk_c")
                    st["wv"] = wp.tile([P, NFD, P], dt_mm, tag="wv", name="wv_c")
                    for key, w in (("wq", wqt), ("wk", wkt), ("wv", wvt)):
                        nc.sync.dma_start(
                            st[key][:],
                            w[:, d0 : d0 + P].rearrange("(f p) d -> p f d", p=P),
                        )
                    st["qraw"] = rawp.tile([P, T], dt_mm, tag="qraw", name="q_raw")
                    st["kraw"] = rawp.tile([P, T], dt_mm, tag="kraw", name="k_raw")
                    st["qraws"] = rawp.tile([P, T], dt_mm, tag="qraws", name="q_raws")
                    st["kraws"] = rawp.tile([P, T], dt_mm, tag="kraws", name="k_raws")
                    st["vt"] = vtp.tile([P, T], dt_mm, tag="vt", name="v_t")

                units.append(u_wdma)

                def u_mm(w_key, nb, f, start, stop):
                    def go():
                        if start:
                            st["ps"] = ps_proj.tile([P, NQ], F32, tag="ps", name="ps")
                        nc.tensor.matmul(
                            st["ps"][:],
                            st[w_key][:, f, :],
                            xt_ts[f][:, nb * NQ : (nb + 1) * NQ],
                            start=start,
                            stop=stop,
                        )

                    return go

                def u_evict(b_t, dst_key, dslice):
                    def go():
                        nc.vector.tensor_scalar_add(
                            st[dst_key][:, dslice],
                            st["ps"][:],
                            b_t[:, pr : pr + 1],
                        )

                    return go

                def u_swap(rkey, skey, nb):
                    def go():
                        sl = slice(nb * NQ, (nb + 1) * NQ)
                        dma_swap32(st[skey][:, sl], st[rkey][:, sl])

                    return go

                for nb in range(T // NQ):
                    for w_key, b_t, dst_key in (
                        ("wk", bk_t, "kraw"),
                        ("wv", bv_t, "vt"),
                        ("wq", bq_t, "qraw"),
                    ):
                        for f in range(NFD):
                            units.append(
                                u_mm(w_key, nb, f, f == 0, f == NFD - 1)
                            )
                        units.append(
                            u_evict(b_t, dst_key, slice(nb * NQ, (nb + 1) * NQ))
                        )
                        if dst_key != "vt":
                            units.append(
                                u_swap(dst_key, dst_key + "s", nb)
                            )

                def u_rope():
                    st["qt"] = qkp.tile([P, T], dt_mm, tag="qt", name="qt")
                    rope(st["qraw"], st["qraws"], csak_t, csbk_t, st["qt"])

                def u_rope2():
                    st["kt"] = qkp.tile([P, T], dt_mm, tag="kt", name="kt")
                    rope(st["kraw"], st["kraws"], csak_t, csbk_t, st["kt"])

                units.append(u_rope)
                units.append(u_rope2)

                def u_vn_alloc(hh):
                    def go():
                        vn_h = vnp.tile(
                            [P, NCH, HD + 1], dt_mm, tag=f"vn{hh}", name="vn_h"
                        )
                        # col 64 = ones (softmax denominator)
                        nc.vector.memset(vn_h[:, :, HD : HD + 1], 1.0)
                        st[f"vn{hh}"] = vn_h

                    return go

                def u_vtr4(hh, g):
                    # transpose 4 key chunks of head hh, one PSUM->SBUF copy
                    def go():
                        tp = ps_proj.tile([P, 4, HD], dt_mm, tag="ps", name="tp")
                        h0 = hh * HD
                        for k in range(4):
                            ch = g * 4 + k
                            nc.tensor.transpose(
                                tp[:, k, :],
                                st["vt"][h0 : h0 + HD, ch * P : (ch + 1) * P],
                                ident[h0 : h0 + HD, :],
                            )
                        nc.vector.tensor_copy(
                            st[f"vn{hh}"][:, g * 4 : g * 4 + 4, :HD], tp[:]
                        )

                    return go

                for hh in range(2):
                    units.append(u_vn_alloc(hh))
                    for g in range(NCH // 4):
                        units.append(u_vtr4(hh, g))
                return st, units

            def pump(units, n):
                for _ in range(n):
                    if units:
                        units.pop(0)()

            def attention(pr, st, next_units, pump_rate, carry_norm,
                          late_units=None):
                """Attention for pair pr using st['qt'/'kt'/'vn*'].  The two
                heads' score matmuls are issued adjacently (concurrent PE
                row-tiles).  po packs both heads' PV accumulation per
                query-block; normalization: bit-trick + one-Newton
                reciprocal (DVE), K=1 f32r broadcast matmul, multiply-as-
                eviction.  Returns the deferred normalize closure for the
                last query block (run by the next pair's attention)."""
                attn_h1 = h1p.tile([HD, T], dt_mm, tag="h1", name="attn_h1")

                def make_norm(po, qb):
                    qs = slice(qb * NQ, (qb + 1) * NQ)
                    # Free po fast: the unnormalized numerators go to SBUF on
                    # the (otherwise idle-at-qb-boundary) ACT engine, and the
                    # denominator reciprocal chain runs on the DVE.  Both are
                    # emitted at qb end; the broadcast + multiply (bcast_mul)
                    # are deferred ~4 chunk iterations so the PE never waits
                    # on the chain.
                    au = smallp.tile([HD, 2, NQ], dt_mm, tag="au", name="au")
                    nc.scalar.copy(au[:], po[0:HD, :, :])
                    den = po[HD : HD + 1, :, :]
                    sd = normp.tile([1, 2, NQ], mybir.dt.int32, tag="sd", name="sd")
                    nc.vector.tensor_scalar(
                        sd[:],
                        den.bitcast(mybir.dt.int32),
                        -1,
                        0x7EF311C3,
                        mybir.AluOpType.mult,
                        mybir.AluOpType.add,
                    )
                    y0 = sd[:].bitcast(F32)
                    t0 = normp.tile([1, 2, NQ], F32, tag="t0", name="t0")
                    nc.vector.tensor_mul(t0[:], den, y0)
                    rec = normp.tile(
                        [1, 2, NQ], mybir.dt.float32r, tag="rec", name="rec"
                    )
                    with nc.allow_low_precision(reason="f32r rec for bcast mm"):
                        nc.vector.scalar_tensor_tensor(
                            rec[:], t0[:], -2.0, y0,
                            mybir.AluOpType.add, mybir.AluOpType.mult,
                        )

                    def bcast_mul():
                        for hh in range(2):
                            pb = ps_sc.tile([HD, NQ], F32, tag="sc", name="pb")
                            nc.tensor.matmul(
                                pb[:],
                                ones64r[:],
                                rec[0:1, hh, :],
                                start=True,
                                stop=True,
                            )
                            recb = smallp.tile(
                                [HD, NQ], F32, tag=f"recb{hh}", name="recb"
                            )
                            nc.vector.tensor_copy(recb[:], pb[:])
                            dst = attn[pr][0:HD, qs] if hh == 0 else attn_h1[:, qs]
                            nc.vector.tensor_mul(
                                dst, au[:, hh, :], recb[:]
                            )
                        # odd head into the pair region per query block so
                        # attn[pr] completes incrementally (cross-quadrant
                        # 32-partition copies)
                        nc.vector.tensor_copy(
                            attn[pr][64:96, qs], attn_h1[0:32, qs]
                        )
                        nc.vector.tensor_copy(
                            attn[pr][96:128, qs], attn_h1[32:64, qs]
                        )

                    return bcast_mul

                for qb in range(NQB):
                    qs = slice(qb * NQ, (qb + 1) * NQ)
                    po = ps_po.tile([P, 2, NQ], F32, tag="po", name="po")
                    pending_pv = None
                    for ci in range(NCH):
                        ps2 = ps_sc.tile([P, 2 * NQ], F32, tag="sc", name="ps2")
                        for hh in range(2):
                            h0 = hh * HD
                            nc.tensor.matmul(
                                ps2[:, hh * NQ : (hh + 1) * NQ],
                                st["kt"][h0 : h0 + HD, ci * P : (ci + 1) * P],
                                st["qt"][h0 : h0 + HD, qs],
                                start=True,
                                stop=True,
                            )
                        pexp = exp_p.tile([P, 2 * NQ], dt_mm, tag="ex", name="pexp")
                        nc.scalar.activation(
                            pexp[:], ps2[:], AF.Exp, scale=float(SCALE)
                        )
                        pump(next_units, pump_rate)
                        if ci == 4 and carry_norm is not None:
                            carry_norm()
                            carry_norm = None
                        if late_units and ci >= 6 and late_units[0][0] < qb:
                            late_units.pop(0)[1]()
                        # PV runs one iteration behind so exp latency is hidden
                        if pending_pv is not None:
                            pending_pv()

                        def make_pv(pexp=pexp, ci=ci, po=po):
                            def go():
                                for hh in range(2):
                                    nc.tensor.matmul(
                                        po[0 : HD + 1, hh, :],
                                        st[f"vn{hh}"][:, ci, :],
                                        pexp[:, hh * NQ : (hh + 1) * NQ],
                                        start=(ci == 0),
                                        stop=(ci == NCH - 1),
                                    )

                            return go

                        pending_pv = make_pv()
                    pending_pv()
                    norm = make_norm(po, qb)
                    if qb < NQB - 1:
                        carry_norm = norm

                return norm

            # ---- output projection resources (prefetched up front; the
            # matmuls are pumped into the last pair's attention) ----
            wop = ctx.enter_context(tc.tile_pool(name="wop", bufs=1))
            outp = ctx.enter_context(tc.tile_pool(name="outp", bufs=2))
            bob_t = persist.tile([P, D], F32, tag="bob_t", name="bob_t")
            nc.sync.dma_start(bob_t[:], bob[:])
            wo_c = []
            for ch in range(NPC):
                wo_ch = wop.tile([P, D], dt_mm, tag=f"wo{ch}", name="wo_ch")
                nc.sync.dma_start(wo_ch[:], wot[ch * P : (ch + 1) * P, :])
                wo_c.append(wo_ch)

            osb_of = {}

            def u_outproj(tb, nh):
                # half an output row-block: 4 accumulating matmuls + bias-sub
                def go():
                    ts = slice(tb * P, (tb + 1) * P)
                    pout = ps_proj.tile([P, NQ], F32, tag="ps", name="pout")
                    for ch in range(NPC):
                        nc.tensor.matmul(
                            pout[:],
                            attn[ch][:, ts],
                            wo_c[ch][:, nh * NQ : (nh + 1) * NQ],
                            start=(ch == 0),
                            stop=(ch == NPC - 1),
                        )
                    if tb not in osb_of:
                        osb_of[tb] = outp.tile([P, D], F32, tag="osb", name="osb")
                    osb = osb_of[tb]
                    # attn tiles carry -attn/den (sign from the Newton
                    # chain); bias - pout restores the sign for free
                    nc.vector.tensor_sub(
                        osb[:, nh * NQ : (nh + 1) * NQ],
                        bob_t[:, nh * NQ : (nh + 1) * NQ],
                        pout[:],
                    )
                    if nh == 1:
                        nc.sync.dma_start(out[ts, :], osb[:])

                return go

            st, units = stage_units(0)
            pump(units, len(units))
            carry_tail = None
            # out-proj row-blocks gated on the attn[NPC-1] query block they
            # read (tb//4); pumped into the last pair's later query blocks
            late_units = [
                (tb // (NQ // P), u_outproj(tb, nh))
                for tb in range(3 * T // P // 4)
                for nh in range(2)
            ]
            for pr in range(NPC):
                if pr + 1 < NPC:
                    nxt_st, nxt_units = stage_units(pr + 1)
                else:
                    nxt_st, nxt_units = None, []
                pump_rate = (len(nxt_units) + 63) // 64 + 1 if nxt_units else 0
                carry_tail = attention(
                    pr, st, nxt_units, pump_rate, carry_tail,
                    late_units=late_units if pr == NPC - 1 else None,
                )
                pump(nxt_units, len(nxt_units))
                st = nxt_st
            carry_tail()
            for _, u in late_units:
                u()
            for tb in range(3 * T // P // 4, T // P):
                for nh in range(2):
                    u_outproj(tb, nh)()

        persist_cm.__exit__(None, None, None)

    _split_sync_waits(nc)
    return nc


# ---------------- host-side input prep ----------------
def _np_dt(dt_mm):
    return ml_dtypes.bfloat16 if dt_mm == mybir.dt.bfloat16 else np.float32


def _cs_tiles(frac_b):
    """csa/csb [128, T] f32 RoPE tiles for one batch (frac_b: [T] f32)."""
    i = np.arange(HALF, dtype=np.float64)
    freq = (ROPE_BASE ** (2.0 * i / HD)).astype(np.float32)  # [32]
    pos = frac_b.astype(np.float32) * np.float32(ROPE_SCALE)
    ang = pos[None, :] / freq[:, None]  # [32, T] f32
    a64 = ang.astype(np.float64)
    cos = np.cos(a64).astype(np.float32)
    sin = np.sin(a64).astype(np.float32)
    csa = np.tile(cos, (4, 1))  # [128, T]
    # csb multiplies the pre-swapped raw (raws[p] = raw[swap32(p)]), so the
    # sign lives at the OUTPUT row: rows 0-31 pair with xr and need -sin,
    # rows 32-63 pair with xl and need +sin
    csb = np.tile(np.concatenate([-sin, sin], axis=0), (2, 1))  # [128, T]
    return np.ascontiguousarray(csa), np.ascontiguousarray(csb)


def make_in_maps(x, frac, Wq, bq, Wk, bk, Wv, bv, Wo, bo, dt_mm=DT_MM):
    npdt = _np_dt(dt_mm)
    wqt = np.ascontiguousarray(Wq.T).astype(npdt)  # [D_in, D_out]
    wkt = np.ascontiguousarray(Wk.T).astype(npdt)
    wvt = np.ascontiguousarray(Wv.T).astype(npdt)
    wot = np.ascontiguousarray(Wo.T).astype(npdt)  # [attn_dim, D_out]
    bq_p = bq.reshape(NFD, P).T.astype(np.float32)  # [128, 8 pairs]
    bk_p = bk.reshape(NFD, P).T.astype(np.float32)
    bv_p = bv.reshape(NFD, P).T.astype(np.float32)
    bob = np.ascontiguousarray(np.tile(bo[None, :], (P, 1))).astype(np.float32)
    zeros_bob = np.zeros_like(bob)
    in_maps = []
    for c in range(N_CORES):
        b, hh2 = c // 2, c % 2
        cols = slice(hh2 * DC, (hh2 + 1) * DC)
        prs = slice(hh2 * NPC, (hh2 + 1) * NPC)
        xt_c = np.ascontiguousarray(x[b].T).astype(npdt)  # [D, T]
        csa, csb = _cs_tiles(frac[b])
        in_maps.append(
            {
                "xt": xt_c,
                "wqt": np.ascontiguousarray(wqt[:, cols]),
                "wkt": np.ascontiguousarray(wkt[:, cols]),
                "wvt": np.ascontiguousarray(wvt[:, cols]),
                "wot": np.ascontiguousarray(wot[cols, :]),
                "bq": np.ascontiguousarray(bq_p[:, prs]),
                "bk": np.ascontiguousarray(bk_p[:, prs]),
                "bv": np.ascontiguousarray(bv_p[:, prs]),
                # bias fed once per batch (even core); odd core gets zeros
                "bob": bob if hh2 == 0 else zeros_bob,
                "csak": csa.astype(npdt),
                "csbk": csb.astype(npdt),
            }
        )
    return in_maps


_NC_CACHE = {}


def _get_nc(dt_mm=DT_MM):
    key = str(dt_mm)
    if key not in _NC_CACHE:
        _NC_CACHE[key] = build_nc(dt_mm)
    return _NC_CACHE[key]


def kernel(x, frac, Wq, bq, Wk, bk, Wv, bv, Wo, bo):
    install_shims()
    from concourse.bass_utils import run_bass_kernel_spmd

    x = np.asarray(x, dtype=np.float32)
    frac = np.asarray(frac, dtype=np.float32)
    args = [np.asarray(a, dtype=np.float32) for a in (Wq, bq, Wk, bk, Wv, bv, Wo, bo)]
    in_maps = make_in_maps(x, frac, *args, dt_mm=DT_MM)
    nc = _get_nc(DT_MM)
    res = run_bass_kernel_spmd(nc, in_maps, list(range(N_CORES)))
    out = np.empty((B, T, D), dtype=np.float32)
    for b in range(B):
        out[b] = res.results[2 * b]["out"] + res.results[2 * b + 1]["out"]
    return out


# revision 22
# speedup vs baseline: 1.0684x; 1.0011x over previous
"""Multi-head attention with fraction-based RoPE ("stoich RoPE") on 8
Trainium2 NeuronCores.

Sharding (v4): each core owns one (batch, head-half) pair -- B=4 batches
x 2 head-halves = 8 shards.  A core projects Q/K/V for its 8 heads over
all 2048 tokens (no redundant work), runs attention for its 4 head
pairs, and computes a PARTIAL output projection (its 512 attn dims of
the 1024-dim contraction).  The host sums the two partials per batch;
the output bias is fed only to the even cores so it is added once.

Device program per core (SPMD):
  - x^T resident in SBUF (8 per-f-chunk tiles, loaded once up front).
  - per head pair (4): projections pumped into the previous pair's
    attention; RoPE via two DVE multiplies with the 32-block partition
    swap done by the DMA engines; V transposed on the PE in groups of 4
    chunks per PSUM->SBUF copy.
  - attention: the two heads' score matmuls are issued adjacently
    (contract=64 -> concurrent PE row-tiles 0/64); one [128,1024] exp
    per key chunk on ACT; PV accumulates both heads in one PSUM tile
    with a ones column for the denominators.
  - softmax normalization: bit-trick + one-Newton reciprocal (-1/den)
    on the DVE, broadcast by a K=1 f32r matmul, multiply-as-eviction;
    the sign is repaid by (bias - pout) in the output projection.
  - output projection pumped into the last pair's attention.
"""

import contextlib
import ctypes
import sys
import types

import numpy as np
import ml_dtypes

import concourse.bass as bass
import concourse.mybir as mybir
import concourse.tile as tile
from concourse.masks import make_identity
from concourse.vector_clock import ScopedClock

# ---------------- problem constants (hardcoded per contract) ----------------
B, T, D = 4, 2048, 1024
H, HD = 16, 64  # heads, head dim
HALF = HD // 2
N_CORES = 8
P = 128
NQ = 512  # moving-dim tile for matmuls
NFD = D // P  # 8 contraction chunks for the projections
NPC = 4  # head pairs per core (8 heads)
DC = D // 2  # per-core projection output width
NCH = T // P  # 16 key chunks
NQB = T // NQ  # 4 query blocks
SCALE = 1.0 / np.sqrt(HD)  # folded into exp()
ROPE_SCALE = 1000.0
ROPE_BASE = 10000.0

F32 = mybir.dt.float32
DT_MM = mybir.dt.bfloat16  # dtype of matmul operands (bfloat16 | float32)

_SO_PATH = "/opt/axon/libaxon_pjrt.so"


# ---------------- axon/NTFF environment shims ----------------
def _ntff_profile_hook():
    try:
        lib = ctypes.CDLL(_SO_PATH)
    except OSError:
        return None
    if not hasattr(lib, "axon_start_nrt_profile"):
        return None
    lib.axon_start_nrt_profile.argtypes = [
        ctypes.POINTER(ctypes.c_int64),
        ctypes.c_size_t,
    ]
    lib.axon_start_nrt_profile.restype = ctypes.c_int64
    lib.axon_stop_nrt_profile.argtypes = [ctypes.c_char_p]
    lib.axon_stop_nrt_profile.restype = ctypes.c_int64

    @contextlib.contextmanager
    def _hook(output_dir, device_ids):
        import jax

        jax.devices()
        if device_ids:
            ids = (ctypes.c_int64 * len(device_ids))(*device_ids)
            rc = lib.axon_start_nrt_profile(ids, len(device_ids))
        else:
            rc = lib.axon_start_nrt_profile(None, 0)
        if rc != 0:
            raise RuntimeError(f"axon_start_nrt_profile rc={rc}")
        try:
            yield
        finally:
            n = lib.axon_stop_nrt_profile(str(output_dir).encode())
            if n < 0:
                raise RuntimeError(f"axon_stop_nrt_profile rc={n}")

    return _hook


def install_shims():
    if "antenv.axon_hooks" not in sys.modules:
        mod = types.ModuleType("antenv.axon_hooks")
        hook = _ntff_profile_hook()
        mod.get_axon_ntff_profile_hook = lambda: hook
        mod.set_axon_ntff_profile_hook = lambda h: None
        sys.modules["antenv.axon_hooks"] = mod
    import concourse.bass_utils as bass_utils

    bass_utils.upload_artifacts = lambda tmpdir: str(tmpdir)

    import os

    if os.environ.get("BASS_LDW_OPT") == "1" and not getattr(
        bass_utils, "_ldw_opt_patched", False
    ):
        orig_run = bass_utils.run_command

        def _run_ldw(argv, **kw):
            argv = [
                "--enable-ldw-opt=true" if a == "--enable-ldw-opt=false" else a
                for a in argv
            ]
            return orig_run(argv, **kw)

        bass_utils.run_command = _run_ldw
        bass_utils._ldw_opt_patched = True


class TileContextSplitDrain(tile.TileContext):
    """This walrus build encodes at most 2 sync waits per CTRL
    instruction; Tile's kernel-tail drain wants one wait per logical
    processor.  Split the waits across single-wait NOPs instead."""

    MAX_WAITS = 1

    def _drain_and_barrier(self, tick_clock, wait_clock):
        nc = self.nc
        carrier = nc.sync.nop(nofuse=True)
        wait_clock.add_sem_waits(
            carrier.ins, ScopedClock({None: tick_clock.global_clock})
        )
        waits = list(carrier.ins.sync_info.on_wait or [])
        if len(waits) > self.MAX_WAITS:
            carrier.ins.sync_info.on_wait[:] = waits[: self.MAX_WAITS]
            for i in range(self.MAX_WAITS, len(waits), self.MAX_WAITS):
                extra = nc.sync.nop(nofuse=True)
                extra.ins.sync_info = mybir.SyncInfo(
                    on_wait=list(waits[i : i + self.MAX_WAITS]), on_update=[]
                )
        nc.sync.drain()
        nc.all_engine_barrier()
        assert self.sems is not None
        popped = nc._tile_sem_poison_stack.pop()
        assert popped is self._sem_poison
        nc.clear_and_free_semaphores(list(self.sems.allocated().values()))
        nc.all_engine_barrier()


def _split_sync_waits(nc, max_waits=1):
    """This walrus build rejects instructions carrying more than ~2 sync
    waits.  Move excess waits onto same-engine NOPs inserted just before
    the instruction (AND semantics are preserved: the engine blocks on
    each carrier in program order)."""
    for f in nc.m.functions:
        for bb in f.blocks:
            out = []
            for inst in bb.instructions:
                si = inst.sync_info
                waits = list(si.on_wait) if si and si.on_wait else []
                if len(waits) > max_waits:
                    for i in range(0, len(waits) - max_waits, max_waits):
                        nop = mybir.InstNoOp(
                            name=nc.get_next_instruction_name(), ins=[], outs=[]
                        )
                        nop.engine = inst.engine
                        nop.sync_info = mybir.SyncInfo(
                            on_wait=list(waits[i : i + max_waits]), on_update=[]
                        )
                        nc.register_instruction(nop, overwrite=True)
                        out.append(nop)
                    si.on_wait[:] = waits[len(waits) - max_waits :]
                out.append(inst)
            bb.instructions[:] = out


# ---------------- device program ----------------
def build_nc(dt_mm=DT_MM):
    nc = bass.Bass(
        "TRN2", target_bir_lowering=False, debug=False, num_devices=N_CORES
    )

    xt = nc.dram_tensor("xt", [D, T], dt_mm, kind="ExternalInput")
    wqt = nc.dram_tensor("wqt", [D, DC], dt_mm, kind="ExternalInput")
    wkt = nc.dram_tensor("wkt", [D, DC], dt_mm, kind="ExternalInput")
    wvt = nc.dram_tensor("wvt", [D, DC], dt_mm, kind="ExternalInput")
    wot = nc.dram_tensor("wot", [DC, D], dt_mm, kind="ExternalInput")
    bq = nc.dram_tensor("bq", [P, NPC], F32, kind="ExternalInput")
    bk = nc.dram_tensor("bk", [P, NPC], F32, kind="ExternalInput")
    bv = nc.dram_tensor("bv", [P, NPC], F32, kind="ExternalInput")
    bob = nc.dram_tensor("bob", [P, D], F32, kind="ExternalInput")
    csak = nc.dram_tensor("csak", [P, T], dt_mm, kind="ExternalInput")
    csbk = nc.dram_tensor("csbk", [P, T], dt_mm, kind="ExternalInput")
    out = nc.dram_tensor("out", [T, D], F32, kind="ExternalOutput")

    AF = mybir.ActivationFunctionType

    with TileContextSplitDrain(nc) as tc:
        persist_cm = tc.tile_pool(name="persist", bufs=1)
        persist = persist_cm.__enter__()

        def ptile(shape, dt, tag):
            return persist.tile(shape, dt, tag=tag, name=tag)

        with contextlib.ExitStack() as ctx:
            # ---- persistent tiles ----
            # resident x^T, one tile per contraction chunk so the first
            # projection can start as soon as chunk 0 lands
            xt_ts = [ptile([P, T], dt_mm, f"xt{f}") for f in range(NFD)]
            csak_t = ptile([P, T], dt_mm, "csak_t")
            csbk_t = ptile([P, T], dt_mm, "csbk_t")
            bq_t = ptile([P, NPC], F32, "bq_t")
            bk_t = ptile([P, NPC], F32, "bk_t")
            bv_t = ptile([P, NPC], F32, "bv_t")
            ident = ptile([P, HD], dt_mm, "ident")
            ones64_f = ptile([1, HD], F32, "ones64_f")
            ones64r = ptile([1, HD], mybir.dt.float32r, "ones64r")
            attn = [ptile([P, T], dt_mm, f"attn{pr}") for pr in range(NPC)]
            for f in range(NFD):
                nc.sync.dma_start(xt_ts[f][:], xt[f * P : (f + 1) * P, :])
            nc.sync.dma_start(csak_t[:], csak[:])
            nc.sync.dma_start(csbk_t[:], csbk[:])
            nc.sync.dma_start(bq_t[:], bq[:])
            nc.sync.dma_start(bk_t[:], bk[:])
            nc.sync.dma_start(bv_t[:], bv[:])
            make_identity(nc, ident[0:HD, :])
            make_identity(nc, ident[HD : 2 * HD, :])
            # +1: the Newton chain yields -1/den, so pb = -1/den and the
            # attn tiles carry -attn/den; the output projection's
            # (bias - pout) restores the sign
            nc.vector.memset(ones64_f[:], 1.0)
            with nc.allow_low_precision(reason="f32r ones for rec bcast"):
                nc.scalar.copy(ones64r[:], ones64_f[:])

            # ---- pools for the head-pair loop ----
            wp = ctx.enter_context(tc.tile_pool(name="wp", bufs=2))
            rawp = ctx.enter_context(tc.tile_pool(name="rawp", bufs=1))
            ropep = ctx.enter_context(tc.tile_pool(name="ropep", bufs=1))
            vtp = ctx.enter_context(tc.tile_pool(name="vtp", bufs=1))
            qkp = ctx.enter_context(tc.tile_pool(name="qkp", bufs=2))
            vnp = ctx.enter_context(tc.tile_pool(name="vnp", bufs=2))
            exp_p = ctx.enter_context(tc.tile_pool(name="exp_p", bufs=4))
            smallp = ctx.enter_context(tc.tile_pool(name="smallp", bufs=2))
            normp = ctx.enter_context(tc.tile_pool(name="normp", bufs=1))
            h1p = ctx.enter_context(tc.tile_pool(name="h1p", bufs=2))
            ps_proj = ctx.enter_context(
                tc.tile_pool(name="ps_proj", bufs=2, space="PSUM")
            )
            ps_sc = ctx.enter_context(
                tc.tile_pool(name="ps_sc", bufs=2, space="PSUM")
            )
            ps_po = ctx.enter_context(
                tc.tile_pool(name="ps_po", bufs=1, space="PSUM")
            )

            def rope(raw, raws, csa_t, csb_t, out_tile):
                # out = raw*csa + swap32(raw)*csb.  The 32-block partition
                # swap is done by the DMA engines (raws), freeing the DVE.
                m1 = ropep.tile([P, NQ], dt_mm, tag="m1", name="m1")
                m2 = ropep.tile([P, NQ], dt_mm, tag="m2", name="m2")
                nc.vector.tensor_mul(m1[:], raw, csa_t)
                nc.vector.tensor_mul(m2[:], raws, csb_t)
                nc.vector.tensor_add(out_tile[:], m1[:], m2[:])

            def dma_swap32(dst, src_t):
                # dst[32-block swapped within each 64-block] = src
                for blk in range(2):
                    b0 = blk * 64
                    nc.sync.dma_start(
                        dst[b0 : b0 + 32, :], src_t[b0 + 32 : b0 + 64, :]
                    )
                    nc.sync.dma_start(
                        dst[b0 + 32 : b0 + 64, :], src_t[b0 : b0 + 32, :]
                    )

            def stage_units(pr):
                """Emission units for pair pr's projections + RoPE + V
                transpose.  Each unit emits a small instruction group; the
                attention loop of the previous pair pumps these so the PE
                stays dense while ACT works on exp."""
                d0 = pr * P
                st = {}
                units = []

                def u_wdma():
                    st["wq"] = wp.tile([P, NFD, P], dt_mm, tag="wq", name="wq_c")
                    st["wk"] = wp.tile([P, NFD, P], dt_mm, tag="wk", name="wk_c")
                    st["wv"] = wp.tile([P, NFD, P], dt_mm, tag="wv", name="wv_c")
                    for key, w in (("wq", wqt), ("wk", wkt), ("wv", wvt)):
                        nc.sync.dma_start(
                            st[key][:],
                            w[:, d0 : d0 + P].rearrange("(f p) d -> p f d", p=P),
                        )
                    st["qraw"] = rawp.tile([P, T], dt_mm, tag="qraw", name="q_raw")
                    st["kraw"] = rawp.tile([P, T], dt_mm, tag="kraw", name="k_raw")
                    st["qraws"] = rawp.tile([P, T], dt_mm, tag="qraws", name="q_raws")
                    st["kraws"] = rawp.tile([P, T], dt_mm, tag="kraws", name="k_raws")
                    st["vt"] = vtp.tile([P, T], dt_mm, tag="vt", name="v_t")

                units.append(u_wdma)

                def u_mm(w_key, nb, f, start, stop):
                    def go():
                        if start:
                            st["ps"] = ps_proj.tile([P, NQ], F32, tag="ps", name="ps")
                        nc.tensor.matmul(
                            st["ps"][:],
                            st[w_key][:, f, :],
                            xt_ts[f][:, nb * NQ : (nb + 1) * NQ],
                            start=start,
                            stop=stop,
                        )

                    return go

                def u_evict(b_t, dst_key, dslice):
                    def go():
                        nc.vector.tensor_scalar_add(
                            st[dst_key][:, dslice],
                            st["ps"][:],
                            b_t[:, pr : pr + 1],
                        )

                    return go

                def u_swap(rkey, skey, nb):
                    def go():
                        sl = slice(nb * NQ, (nb + 1) * NQ)
                        dma_swap32(st[skey][:, sl], st[rkey][:, sl])

                    return go

                def u_rope(rkey, dst_list, nb):
                    # per-chunk RoPE into its own tile, so the first
                    # attention chunks can start before the whole stage ran
                    def go():
                        tile_ = qkp.tile(
                            [P, NQ], dt_mm, tag=f"{dst_list}{nb}", name=dst_list
                        )
                        sl = slice(nb * NQ, (nb + 1) * NQ)
                        rope(
                            st[rkey][:, sl],
                            st[rkey + "s"][:, sl],
                            csak_t[:, sl],
                            csbk_t[:, sl],
                            tile_,
                        )
                        st[dst_list][nb] = tile_

                    return go

                def u_vn_alloc(hh, g):
                    def go():
                        vn_g = vnp.tile(
                            [P, 4, HD + 1], dt_mm, tag=f"vn{hh}g{g}", name="vn_g"
                        )
                        # col 64 = ones (softmax denominator)
                        nc.vector.memset(vn_g[:, :, HD : HD + 1], 1.0)
                        st[f"vn{hh}"][g] = vn_g

                    return go

                def u_vtr4(hh, g):
                    # transpose 4 key chunks of head hh, one PSUM->SBUF copy
                    def go():
                        tp = ps_proj.tile([P, 4, HD], dt_mm, tag="ps", name="tp")
                        h0 = hh * HD
                        for k in range(4):
                            ch = g * 4 + k
                            nc.tensor.transpose(
                                tp[:, k, :],
                                st["vt"][h0 : h0 + HD, ch * P : (ch + 1) * P],
                                ident[h0 : h0 + HD, :],
                            )
                        nc.vector.tensor_copy(
                            st[f"vn{hh}"][g][:, :, :HD], tp[:]
                        )

                    return go

                st["kt"] = [None] * (T // NQ)
                st["qt"] = [None] * (T // NQ)
                st["vn0"] = [None] * (NCH // 4)
                st["vn1"] = [None] * (NCH // 4)

                def proj_group(w_key, b_t, dst_key, nb):
                    g = [
                        u_mm(w_key, nb, f, f == 0, f == NFD - 1)
                        for f in range(NFD)
                    ]
                    g.append(
                        u_evict(b_t, dst_key, slice(nb * NQ, (nb + 1) * NQ))
                    )
                    return g

                # consumption order: attention chunk ci needs kt[ci//4] and
                # vn group ci//4; query block qb needs qt[qb]
                for nb in range(T // NQ):
                    units += proj_group("wk", bk_t, "kraw", nb)
                    units.append(u_swap("kraw", "kraws", nb))
                    units.append(u_rope("kraw", "kt", nb))
                    units += proj_group("wv", bv_t, "vt", nb)
                    for hh in range(2):
                        units.append(u_vn_alloc(hh, nb))
                        units.append(u_vtr4(hh, nb))
                    units += proj_group("wq", bq_t, "qraw", nb)
                    units.append(u_swap("qraw", "qraws", nb))
                    units.append(u_rope("qraw", "qt", nb))
                return st, units

            def pump(units, n):
                for _ in range(n):
                    if units:
                        units.pop(0)()

            def attention(pr, st, next_units, pump_rate, carry_norm,
                          late_units=None):
                """Attention for pair pr using st['qt'/'kt'/'vn*'].  The two
                heads' score matmuls are issued adjacently (concurrent PE
                row-tiles).  po packs both heads' PV accumulation per
                query-block; normalization: bit-trick + one-Newton
                reciprocal (DVE), K=1 f32r broadcast matmul, multiply-as-
                eviction.  Returns the deferred normalize closure for the
                last query block (run by the next pair's attention)."""
                attn_h1 = h1p.tile([HD, T], dt_mm, tag="h1", name="attn_h1")

                def make_norm(po, qb):
                    qs = slice(qb * NQ, (qb + 1) * NQ)
                    # Free po fast: the unnormalized numerators go to SBUF on
                    # the (otherwise idle-at-qb-boundary) ACT engine, and the
                    # denominator reciprocal chain runs on the DVE.  Both are
                    # emitted at qb end; the broadcast + multiply (bcast_mul)
                    # are deferred ~4 chunk iterations so the PE never waits
                    # on the chain.
                    au = smallp.tile([HD, 2, NQ], dt_mm, tag="au", name="au")
                    nc.scalar.copy(au[:], po[0:HD, :, :])
                    den = po[HD : HD + 1, :, :]
                    sd = normp.tile([1, 2, NQ], mybir.dt.int32, tag="sd", name="sd")
                    nc.vector.tensor_scalar(
                        sd[:],
                        den.bitcast(mybir.dt.int32),
                        -1,
                        0x7EF311C3,
                        mybir.AluOpType.mult,
                        mybir.AluOpType.add,
                    )
                    y0 = sd[:].bitcast(F32)
                    t0 = normp.tile([1, 2, NQ], F32, tag="t0", name="t0")
                    nc.vector.tensor_mul(t0[:], den, y0)
                    rec = normp.tile(
                        [1, 2, NQ], mybir.dt.float32r, tag="rec", name="rec"
                    )
                    with nc.allow_low_precision(reason="f32r rec for bcast mm"):
                        nc.vector.scalar_tensor_tensor(
                            rec[:], t0[:], -2.0, y0,
                            mybir.AluOpType.add, mybir.AluOpType.mult,
                        )

                    def bcast_mul():
                        for hh in range(2):
                            pb = ps_sc.tile([HD, NQ], F32, tag="sc", name="pb")
                            nc.tensor.matmul(
                                pb[:],
                                ones64r[:],
                                rec[0:1, hh, :],
                                start=True,
                                stop=True,
                            )
                            recb = smallp.tile(
                                [HD, NQ], F32, tag=f"recb{hh}", name="recb"
                            )
                            nc.vector.tensor_copy(recb[:], pb[:])
                            dst = attn[pr][0:HD, qs] if hh == 0 else attn_h1[:, qs]
                            nc.vector.tensor_mul(
                                dst, au[:, hh, :], recb[:]
                            )
                        # odd head into the pair region per query block so
                        # attn[pr] completes incrementally (cross-quadrant
                        # 32-partition copies)
                        nc.vector.tensor_copy(
                            attn[pr][64:96, qs], attn_h1[0:32, qs]
                        )
                        nc.vector.tensor_copy(
                            attn[pr][96:128, qs], attn_h1[32:64, qs]
                        )

                    return bcast_mul

                for qb in range(NQB):
                    qs = slice(qb * NQ, (qb + 1) * NQ)
                    po = ps_po.tile([P, 2, NQ], F32, tag="po", name="po")
                    pending_pv = None
                    for ci in range(NCH):
                        # on-demand pump until this chunk's tiles exist
                        # (only triggers while attention(0) is still owed
                        # the tail of stage 0)
                        while next_units and (
                            st["kt"][ci // 4] is None
                            or st["qt"][qb] is None
                            or st["vn0"][ci // 4] is None
                            or st["vn1"][ci // 4] is None
                        ):
                            pump(next_units, 1)
                        ps2 = ps_sc.tile([P, 2 * NQ], F32, tag="sc", name="ps2")
                        kt_c = st["kt"][ci // 4]
                        qt_b = st["qt"][qb]
                        co = (ci % 4) * P
                        for hh in range(2):
                            h0 = hh * HD
                            nc.tensor.matmul(
                                ps2[:, hh * NQ : (hh + 1) * NQ],
                                kt_c[h0 : h0 + HD, co : co + P],
                                qt_b[h0 : h0 + HD, :],
                                start=True,
                                stop=True,
                            )
                        pexp = exp_p.tile([P, 2 * NQ], dt_mm, tag="ex", name="pexp")
                        nc.scalar.activation(
                            pexp[:], ps2[:], AF.Exp, scale=float(SCALE)
                        )
                        pump(next_units, pump_rate)
                        if ci == 4 and carry_norm is not None:
                            carry_norm()
                            carry_norm = None
                        if late_units and ci >= 6 and late_units[0][0] < qb:
                            late_units.pop(0)[1]()
                        # PV runs one iteration behind so exp latency is hidden
                        if pending_pv is not None:
                            pending_pv()

                        def make_pv(pexp=pexp, ci=ci, po=po):
                            def go():
                                for hh in range(2):
                                    nc.tensor.matmul(
                                        po[0 : HD + 1, hh, :],
                                        st[f"vn{hh}"][ci // 4][:, ci % 4, :],
                                        pexp[:, hh * NQ : (hh + 1) * NQ],
                                        start=(ci == 0),
                                        stop=(ci == NCH - 1),
                                    )

                            return go

                        pending_pv = make_pv()
                    pending_pv()
                    norm = make_norm(po, qb)
                    if qb < NQB - 1:
                        carry_norm = norm

                return norm

            # ---- output projection resources (prefetched up front; the
            # matmuls are pumped into the last pair's attention) ----
            wop = ctx.enter_context(tc.tile_pool(name="wop", bufs=1))
            outp = ctx.enter_context(tc.tile_pool(name="outp", bufs=2))
            bob_t = persist.tile([P, D], F32, tag="bob_t", name="bob_t")
            nc.sync.dma_start(bob_t[:], bob[:])
            wo_c = []
            for ch in range(NPC):
                wo_ch = wop.tile([P, D], dt_mm, tag=f"wo{ch}", name="wo_ch")
                nc.sync.dma_start(wo_ch[:], wot[ch * P : (ch + 1) * P, :])
                wo_c.append(wo_ch)

            osb_of = {}

            def u_outproj(tb, nh):
                # half an output row-block: 4 accumulating matmuls + bias-sub
                def go():
                    ts = slice(tb * P, (tb + 1) * P)
                    pout = ps_proj.tile([P, NQ], F32, tag="ps", name="pout")
                    for ch in range(NPC):
                        nc.tensor.matmul(
                            pout[:],
                            attn[ch][:, ts],
                            wo_c[ch][:, nh * NQ : (nh + 1) * NQ],
                            start=(ch == 0),
                            stop=(ch == NPC - 1),
                        )
                    if tb not in osb_of:
                        osb_of[tb] = outp.tile([P, D], F32, tag="osb", name="osb")
                    osb = osb_of[tb]
                    # attn tiles carry -attn/den (sign from the Newton
                    # chain); bias - pout restores the sign for free
                    nc.vector.tensor_sub(
                        osb[:, nh * NQ : (nh + 1) * NQ],
                        bob_t[:, nh * NQ : (nh + 1) * NQ],
                        pout[:],
                    )
                    if nh == 1:
                        nc.sync.dma_start(out[ts, :], osb[:])

                return go

            st, units = stage_units(0)
            # flush units for token-block 0 (enough for attention chunk 0);
            # the rest of stage 0 is pumped by attention(0) itself (with
            # on-demand pumping when a needed tile is not yet emitted)
            pump(units, 35)
            carry_tail = None
            # out-proj row-blocks gated on the attn[NPC-1] query block they
            # read (tb//4); pumped into the last pair's later query blocks
            late_units = [
                (tb // (NQ // P), u_outproj(tb, nh))
                for tb in range(3 * T // P // 4)
                for nh in range(2)
            ]
            for pr in range(NPC):
                if pr + 1 < NPC:
                    nxt_st, nxt_units = stage_units(pr + 1)
                else:
                    nxt_st, nxt_units = None, []
                # attention(0) still owes the tail of stage 0: pump it first
                pump_units = units + nxt_units if pr == 0 else nxt_units
                pump_rate = (len(pump_units) + 63) // 64 + 1 if pump_units else 0
                carry_tail = attention(
                    pr, st, pump_units, pump_rate, carry_tail,
                    late_units=late_units if pr == NPC - 1 else None,
                )
                pump(pump_units, len(pump_units))
                st = nxt_st
            carry_tail()
            for _, u in late_units:
                u()
            for tb in range(3 * T // P // 4, T // P):
                for nh in range(2):
                    u_outproj(tb, nh)()

        persist_cm.__exit__(None, None, None)

    _split_sync_waits(nc)
    return nc


# ---------------- host-side input prep ----------------
def _np_dt(dt_mm):
    return ml_dtypes.bfloat16 if dt_mm == mybir.dt.bfloat16 else np.float32


def _cs_tiles(frac_b):
    """csa/csb [128, T] f32 RoPE tiles for one batch (frac_b: [T] f32)."""
    i = np.arange(HALF, dtype=np.float64)
    freq = (ROPE_BASE ** (2.0 * i / HD)).astype(np.float32)  # [32]
    pos = frac_b.astype(np.float32) * np.float32(ROPE_SCALE)
    ang = pos[None, :] / freq[:, None]  # [32, T] f32
    a64 = ang.astype(np.float64)
    cos = np.cos(a64).astype(np.float32)
    sin = np.sin(a64).astype(np.float32)
    csa = np.tile(cos, (4, 1))  # [128, T]
    # csb multiplies the pre-swapped raw (raws[p] = raw[swap32(p)]), so the
    # sign lives at the OUTPUT row: rows 0-31 pair with xr and need -sin,
    # rows 32-63 pair with xl and need +sin
    csb = np.tile(np.concatenate([-sin, sin], axis=0), (2, 1))  # [128, T]
    return np.ascontiguousarray(csa), np.ascontiguousarray(csb)


def make_in_maps(x, frac, Wq, bq, Wk, bk, Wv, bv, Wo, bo, dt_mm=DT_MM):
    npdt = _np_dt(dt_mm)
    wqt = np.ascontiguousarray(Wq.T).astype(npdt)  # [D_in, D_out]
    wkt = np.ascontiguousarray(Wk.T).astype(npdt)
    wvt = np.ascontiguousarray(Wv.T).astype(npdt)
    wot = np.ascontiguousarray(Wo.T).astype(npdt)  # [attn_dim, D_out]
    bq_p = bq.reshape(NFD, P).T.astype(np.float32)  # [128, 8 pairs]
    bk_p = bk.reshape(NFD, P).T.astype(np.float32)
    bv_p = bv.reshape(NFD, P).T.astype(np.float32)
    bob = np.ascontiguousarray(np.tile(bo[None, :], (P, 1))).astype(np.float32)
    zeros_bob = np.zeros_like(bob)
    in_maps = []
    for c in range(N_CORES):
        b, hh2 = c // 2, c % 2
        cols = slice(hh2 * DC, (hh2 + 1) * DC)
        prs = slice(hh2 * NPC, (hh2 + 1) * NPC)
        xt_c = np.ascontiguousarray(x[b].T).astype(npdt)  # [D, T]
        csa, csb = _cs_tiles(frac[b])
        in_maps.append(
            {
                "xt": xt_c,
                "wqt": np.ascontiguousarray(wqt[:, cols]),
                "wkt": np.ascontiguousarray(wkt[:, cols]),
                "wvt": np.ascontiguousarray(wvt[:, cols]),
                "wot": np.ascontiguousarray(wot[cols, :]),
                "bq": np.ascontiguousarray(bq_p[:, prs]),
                "bk": np.ascontiguousarray(bk_p[:, prs]),
                "bv": np.ascontiguousarray(bv_p[:, prs]),
                # bias fed once per batch (even core); odd core gets zeros
                "bob": bob if hh2 == 0 else zeros_bob,
                "csak": csa.astype(npdt),
                "csbk": csb.astype(npdt),
            }
        )
    return in_maps


_NC_CACHE = {}


def _get_nc(dt_mm=DT_MM):
    key = str(dt_mm)
    if key not in _NC_CACHE:
        _NC_CACHE[key] = build_nc(dt_mm)
    return _NC_CACHE[key]


def kernel(x, frac, Wq, bq, Wk, bk, Wv, bv, Wo, bo):
    install_shims()
    from concourse.bass_utils import run_bass_kernel_spmd

    x = np.asarray(x, dtype=np.float32)
    frac = np.asarray(frac, dtype=np.float32)
    args = [np.asarray(a, dtype=np.float32) for a in (Wq, bq, Wk, bk, Wv, bv, Wo, bo)]
    in_maps = make_in_maps(x, frac, *args, dt_mm=DT_MM)
    nc = _get_nc(DT_MM)
    res = run_bass_kernel_spmd(nc, in_maps, list(range(N_CORES)))
    out = np.empty((B, T, D), dtype=np.float32)
    for b in range(B):
        out[b] = res.results[2 * b]["out"] + res.results[2 * b + 1]["out"]
    return out
